# revision 96
# baseline (speedup 1.0000x reference)
"""AdaptiveSparseAttention Trainium2 kernel (8-core head-parallel).

Problem: B=1, H=16, S=2048, D=128 fp32, causal attention with an adaptive
block mask: mean-pool Q/K per 64-block, softmax block scores, keep the
minimal top-p (0.95) set of key blocks per query block (plus diagonal).

Sharding: 2 heads per NeuronCore, fully local (no collectives).

Host-side prep (numpy, outside the NEFF): q,k shipped pre-transposed
[D, S] bf16; v shipped bf16 pre-chunked into the SBUF tile layout with a
ones column appended (so P@V produces the softmax denominator for free);
per-head Q/K block sums (with the k sequence-mean removed -- smooth_k)
and all static mask constants packed into two small tensors; the final
numerator/denominator divide and output transpose also run on the host.
smooth_k is dropped from the main logits path entirely: subtracting the
per-head K mean shifts every logit of a softmax row by a per-query
constant, which softmax cancels exactly.

Device program (per head, heads sequential through one software-
pipelined wave loop):
  - [32,32] f32 block-score chain reproduces the reference
    argsort/cumsum top-p construction exactly (no ties): causal-masked
    block logits, unnormalized softmax (fixed e^-9 shift -- the top-p
    ratio test is scale-invariant), pairwise strictly-greater mass
    comparison against tau*rowsum, diagonal forced on.  The keep mask is
    expanded to negk[kb, qi] (-1e9 where dropped, bf16, 128 partitions
    with ones-padded keepT so rows 32:128 are exact zeros).
  - flash attention over groups of 512 queries in [128, 1024] f32 psum
    waves (2 kj-chunks x 512 queries): per chunk ONE QK matmul
    LT[kj,qi] = kT.T @ qT (bf16, leading above-diagonal 128-blocks
    skipped) immediately followed by the block-mask add as a second
    matmul (indall.T @ negk, K=128 so LDWEIGHTS pipelining never
    alternates); token-level causal via [128,128] triangular DVE adds on
    diagonal chunks; exp on ScalarE (scale=1/sqrt(D), bias=-9) -> PT
    bf16; P@V accumulates two full-bank [128,512] psum tiles per group
    (2x 129 cols each; only the very first matmul into a tile sets
    start=True -- start clears has_written for the WHOLE bank).
  - emission is software-pipelined two waves ahead (QK of wave i+2
    issues right after exp of wave i) so the PE streams back-to-back;
    input DMAs are split across both hwdge rings (sync + scalar) in
    first-use order, with bulk transfers dispatched after the head-0
    mask chain so they never delay it.
"""

import math
import threading

import numpy as np

_B, _H, _S, _D = 1, 16, 2048, 128
_NCORES = 8
_HLOC = _H // _NCORES  # heads per core
_BLK = 64
_NB = _S // _BLK       # 32 key/query blocks
_TAU = 0.95
_SCALE = 1.0 / math.sqrt(_D)
_SHIFT = 9.0           # constant softmax shift; |scaled logits| < ~6
_BIGM = 1.0e9          # additive mask magnitude (pre-scale), diag tri only
_NEG_BL = -1.0e30      # block-logit causal mask value (matches reference)

_NCHUNK = _S // 128    # 16 sequence chunks of 128


class _Head:
    pass


def _emit(nc, tc, pools, consts, qT_d, kT_d, va_d, out_d, mybir):
    f32 = mybir.dt.float32
    bf16 = mybir.dt.bfloat16
    AF = mybir.ActivationFunctionType
    OP = mybir.AluOpType
    AX = mybir.AxisListType

    psP = pools["psP"]
    psM = pools["psM"]
    big = pools["big"]
    sm = pools["sm"]
    ptp = pools["ptp"]
    outp = pools["outp"]

    indall = consts["indall"]
    tri128 = consts["tri128"]
    causal_add = consts["causal_add"]
    causal01 = consts["causal01"]
    eye01 = consts["eye01"]
    nshift = consts["nshift"]

    heads = []
    for h in range(_HLOC):
        H = _Head()
        H.h = h
        H.qT = big.tile([128, _S], bf16, tag="qT", name=f"qT{h}")
        H.kT = big.tile([128, _S], bf16, tag="kT", name=f"kT{h}")
        H.va = big.tile([128, _NCHUNK * 129], bf16, tag="va", name=f"va{h}")
        H.va3 = H.va[:].rearrange("p (c x) -> p c x", x=129)
        heads.append(H)

    # ---- input DMAs (sync ring).  The tiny block-sum tensors go FIRST
    # so the mask chain (which gates the first wave's mask matmul) starts
    # immediately; then head0 k/q (first QK waves), head0 v (first PV),
    # then head1.
    for H in heads:
        h = H.h
        H.qbT = consts["csts"][:, 225 + h * 32:225 + h * 32 + 32]
        H.kbs = consts["csts"][:, 289 + h * 32:289 + h * 32 + 32]
    # Two DMA rings (sync + scalar), transfers ordered by first use so the
    # wave pipeline ramps while the bulk of the input still streams in.
    # Only the head-0 critical pieces are dispatched before the chain --
    # scalar-ring dispatches occupy the ACT queue and must not delay the
    # chain's block-softmax exp.
    H0, H1 = heads
    nc.sync.dma_start(H0.kT[:, 0:1024], kT_d[0][:, 0:1024])
    nc.scalar.dma_start(H0.qT[:, 0:1024], qT_d[0][:, 0:1024])
    nc.sync.dma_start(indall[:], consts["cbf_d"])
    nc.scalar.dma_start(H0.va[:, 0:8 * 129], va_d[0][:, 0:8 * 129])
    nc.sync.dma_start(H0.kT[:, 1024:2048], kT_d[0][:, 1024:2048])

    def emit_bulk_dma():
        nc.scalar.dma_start(H0.qT[:, 1024:2048], qT_d[0][:, 1024:2048])
        nc.scalar.dma_start(H0.va[:, 8 * 129:16 * 129],
                            va_d[0][:, 8 * 129:16 * 129])
        nc.sync.dma_start(H1.kT[:], kT_d[1])
        nc.scalar.dma_start(H1.qT[:], qT_d[1])
        nc.sync.dma_start(H1.va[:], va_d[1])

    # ---- block-score keep chain, split into parts for sprinkling ----
    # qbT ([128,32] f32 q block sums) and kbs (smoothed k block sums)
    # arrive precomputed from the host
    def chain_b(H):
        # blp lives in the acc pool so it never occupies an LT wave slot
        h = H.h
        H.blp = psM.tile([32, 32], f32, tag="acc", name=f"blp{h}")
        nc.tensor.matmul(H.blp[:], H.qbT[:], H.kbs[:], start=True, stop=True)

    def chain_c(H):
        h = H.h
        bl = sm.tile([32, 32], f32, tag="bl", name=f"bl{h}")
        nc.vector.scalar_tensor_tensor(
            bl[:], H.blp[:], _SCALE / float(_BLK * _BLK), causal_add,
            op0=OP.mult, op1=OP.add)
        # unnormalized block softmax with a fixed shift: block logits are
        # bounded (|bl| << 1), and the keep test below compares the sum of
        # strictly-greater exps against tau * rowsum -- invariant to the
        # uniform e^-9 scaling
        H.bp = sm.tile([32, 32], f32, tag="bp", name=f"bp{h}")
        rs = sm.tile([32, 1], f32, tag="rs", name=f"rs{h}")
        nc.scalar.activation(H.bp[:], bl[:], AF.Exp, bias=nshift[0:32],
                             scale=1.0, accum_out=rs[:])
        H.taurs = sm.tile([32, 1], f32, tag="taurs", name=f"taurs{h}")
        nc.vector.tensor_scalar_mul(H.taurs[:], rs[:], _TAU)

    def chain_d(H):
        h = H.h
        a_ap = H.bp[:].unsqueeze(1).broadcast_to((32, 32, 32))
        b_ap = H.bp[:].unsqueeze(2).broadcast_to((32, 32, 32))
        gt = sm.tile([32, 32 * 32], f32, tag="gt", name=f"gt{h}")
        H.gt3 = gt[:].rearrange("p (a b) -> p a b", a=32)
        nc.vector.tensor_tensor(H.gt3, a_ap, b_ap, op=OP.is_gt)

    def chain_e(H):
        h = H.h
        a_ap = H.bp[:].unsqueeze(1).broadcast_to((32, 32, 32))
        pr = sm.tile([32, 32 * 32], f32, tag="pr", name=f"pr{h}")
        pr3 = pr[:].rearrange("p (a b) -> p a b", a=32)
        nc.vector.tensor_tensor(pr3, H.gt3, a_ap, op=OP.mult)
        H.tt = sm.tile([32, 32], f32, tag="tt", name=f"tt{h}")
        nc.vector.reduce_sum(H.tt[:], pr3, axis=AX.X)

    def chain_f(H):
        h = H.h
        keep = sm.tile([32, 32], f32, tag="keep", name=f"keep{h}")
        nc.vector.scalar_tensor_tensor(
            keep[:], H.tt[:], H.taurs[:], causal01,
            op0=OP.is_lt, op1=OP.mult)
        nc.vector.tensor_tensor(keep[:], keep[:], eye01, op=OP.max)
        # keepT padded to 128 partitions with ones (rows 32:128 -> negk 0)
        H.keepT = sm.tile([128, 32], f32, tag="keepT", name=f"keepT{h}")
        nc.vector.memset(H.keepT[32:64, :], 1.0)
        nc.vector.memset(H.keepT[64:96, :], 1.0)
        nc.vector.memset(H.keepT[96:128, :], 1.0)
        nc.vector.transpose(H.keepT[0:32, :], keep[:])

    def sel_dve(g, w0, ci):
        # offloading mask adds to DVE balances engine busy-time on paper
        # (PE 49 / ACT 48 / DVE 46) but couples exp behind the in-order
        # DVE queue and loses ~3us to per-wave stalls -- keep all on PE
        return False

    def chain_i(H):
        # kexp[p, ci*32+qb] = -BIGM where block (qb, kb=2ci+p//64) dropped
        # (partition expansion of the keep mask via tiny K=128 matmuls
        # against the static indicator; keepm1 rows 32:128 are zero)
        h = H.h
        keepm1 = sm.tile([128, 32], bf16, tag="keepm1", name=f"keepm1{h}")
        nc.vector.tensor_scalar(keepm1[:], H.keepT[:], 1.0, _BIGM,
                                op0=OP.subtract, op1=OP.mult)
        kx = psP.tile([128, 512], f32, tag="lt", name=f"kx{h}")
        for ci in range(_NCHUNK):
            nc.tensor.matmul(
                kx[:, ci * 32:(ci + 1) * 32],
                indall[:, ci * 128:(ci + 1) * 128],
                keepm1[:], start=True, stop=True)
        H.kexp = sm.tile([128, _NCHUNK * 32], bf16, tag="kexp",
                         name=f"kexp{h}")
        nc.vector.tensor_copy(H.kexp[:], kx[:])

    def chain_g(H):
        # negk[kb, qi] = -BIGM where block (qb(qi), kb) is dropped, else 0.
        # Full 128 partitions (rows >= 32 zero, from the ones-padding of
        # keepT) so the mask matmul has K=128 like every other main-loop
        # matmul -- alternating K breaks LDWEIGHTS pipelining.
        # built in two pieces: the first 512 query cols (group 0) on DVE so
        # wave 0's mask matmul can start immediately; the rest on gpsimd in
        # parallel
        h = H.h
        H.negk = sm.tile([128, _S], bf16, tag="negk", name=f"negk{h}")
        nc.vector.tensor_scalar(
            H.negk[:, 0:512].rearrange("p (a b) -> p a b", b=_BLK),
            H.keepT[:, 0:8].unsqueeze(2).broadcast_to((128, 8, _BLK)),
            1.0, _BIGM, op0=OP.subtract, op1=OP.mult)
        nc.vector.tensor_scalar(
            H.negk[:, 512:2048].rearrange("p (a b) -> p a b", b=_BLK),
            H.keepT[:, 8:32].unsqueeze(2).broadcast_to((128, 24, _BLK)),
            1.0, _BIGM, op0=OP.subtract, op1=OP.mult)

    # head0's chain runs to completion FIRST (it gates the first mask
    # matmuls); the bulk DMAs and head1's chain follow
    for fn in (chain_b, chain_c, chain_d, chain_e, chain_f, chain_g):
        fn(H0)
    emit_bulk_dma()
    chain_b(H1)

    # ---- flat software-pipelined wave list over both heads ----
    # groups of 512 queries (4 output t-blocks), waves of 2 kj-chunks
    ngrp = _S // 512
    waves = []  # (H, g, w0, wn)
    for H in heads:
        for g in range(ngrp):
            nch = 4 * g + 4
            for w0 in range(0, nch, 2):
                waves.append((H, g, w0, min(2, nch - w0)))

    def emit_qk(i):
        H, g, w0, wn = waves[i]
        qlo = g * 512
        ltw = psP.tile([128, 1024], f32, tag="lt",
                       name=f"lt{H.h}_{g}_{w0}")
        H.ltw = ltw
        for ci in range(w0, w0 + wn):
            # leading 128-col blocks above the causal diagonal are never
            # consumed -- skip them
            qo = max(0, (ci - 4 * g)) * 128
            sl = ltw[:, (ci - w0) * 512 + qo:(ci - w0) * 512 + 512]
            dve = sel_dve(g, w0, ci)
            nc.tensor.matmul(
                sl, H.kT[:, ci * 128:(ci + 1) * 128],
                H.qT[:, qlo + qo:qlo + 512], start=True, stop=dve)
            if not dve:
                nc.tensor.matmul(
                    sl, indall[:, ci * 128:(ci + 1) * 128],
                    H.negk[:, qlo + qo:qlo + 512], start=False, stop=True,
                    skip_group_check=True)
        return ltw

    lt_of = {}
    lt_of[0] = emit_qk(0)
    lt_of[1] = emit_qk(1)
    for fn in (chain_c, chain_d, chain_e, chain_f, chain_g):
        fn(H1)

    for i, (H, g, w0, wn) in enumerate(waves):
        ltw = lt_of.pop(i)
        h = H.h
        # token-level causal on diagonal chunks
        for ci in range(max(w0, 4 * g), w0 + wn):
            off = (ci - w0) * 512 + (ci - 4 * g) * 128
            nc.vector.tensor_tensor(
                ltw[:, off:off + 128], ltw[:, off:off + 128],
                tri128, op=OP.add)
        # DVE-side block-mask adds for the offloaded chunks
        for ci in range(w0, w0 + wn):
            if not sel_dve(g, w0, ci):
                continue
            sl3 = ltw[:, (ci - w0) * 512:(ci - w0) * 512 + 512].rearrange(
                "p (a b) -> p a b", b=_BLK)
            kb = H.kexp[:, ci * 32 + 8 * g:ci * 32 + 8 * g + 8]
            nc.vector.tensor_tensor(
                sl3, sl3, kb.unsqueeze(2).broadcast_to((128, 8, _BLK)),
                op=OP.add)
        # QK of wave i+2 issues before exp(i) in queue order: its psum
        # slot was already freed by exp(i-1), so PE starts it while ACT
        # works through exp(i)
        if i + 2 < len(waves):
            lt_of[i + 2] = emit_qk(i + 2)
        ptw = ptp.tile([128, 1024], bf16, tag="pt",
                       name=f"pt{h}_{g}_{w0}")
        nc.scalar.activation(ptw[:, 0:wn * 512], ltw[:, 0:wn * 512],
                             AF.Exp, bias=nshift, scale=_SCALE)
        if w0 == 0:
            # two full-bank accumulator tiles per 512-query group (psum
            # bank = 512 f32); start=True clears has_written for the
            # WHOLE bank, so only the very first matmul into each tile
            # may set it
            H.acc_lo = psM.tile([128, 512], f32, tag="acc",
                                name=f"accL{h}_{g}")
            H.acc_hi = psM.tile([128, 512], f32, tag="acc",
                                name=f"accH{h}_{g}")
        for ci in range(w0, w0 + wn):
            for t in range(max(4 * g, ci), 4 * g + 4):
                ts = t - 4 * g
                acc = H.acc_lo if ts < 2 else H.acc_hi
                nc.tensor.matmul(
                    acc[:, (ts % 2) * 129:(ts % 2) * 129 + 129],
                    ptw[:, (ci - w0) * 512 + ts * 128:
                        (ci - w0) * 512 + ts * 128 + 128],
                    H.va3[:, ci, :],
                    start=(ci == 0 and ts % 2 == 0),
                    stop=(ci == 4 * g + 2 * (ts // 2) + 1),
                    skip_group_check=True)
        # ship raw numerator+denominator as soon as each accumulator pair
        # stops; the host does the divide.  acc_lo stops with chunk 4g+1.
        if w0 <= 4 * g + 1 < w0 + wn:
            o = outp.tile([128, 258], f32, tag="o", name=f"o{h}_{g}_0")
            nc.vector.tensor_copy(o[:], H.acc_lo[:, 0:258])
            nc.sync.dma_start(out_d[h, 2 * g], o[:])
        if w0 + wn == 4 * g + 4:
            o = outp.tile([128, 258], f32, tag="o", name=f"o{h}_{g}_1")
            nc.vector.tensor_copy(o[:], H.acc_hi[:, 0:258])
            nc.sync.dma_start(out_d[h, 2 * g + 1], o[:])


def build_nc():
    import concourse.mybir as mybir
    import concourse.tile as tile
    from concourse import bacc
    from concourse.masks import make_identity

    f32 = mybir.dt.float32
    bf16 = mybir.dt.bfloat16
    OP = mybir.AluOpType

    nc = bacc.Bacc("TRN2", target_bir_lowering=False, debug=False,
                   enable_asserts=False, num_devices=_NCORES)
    qT_d = nc.dram_tensor("qT", [_HLOC, _D, _S], bf16,
                          kind="ExternalInput").ap()
    kT_d = nc.dram_tensor("kT", [_HLOC, _D, _S], bf16,
                          kind="ExternalInput").ap()
    va_d = nc.dram_tensor("va", [_HLOC, 128, _NCHUNK * 129], bf16,
                          kind="ExternalInput").ap()
    # one packed f32 tensor: consts [128,225] + qbT [2,128,32] + kbs
    # [2,128,32] -> a single small DMA dispatch
    cf32_d = nc.dram_tensor("cf32", [128, 353], f32,
                            kind="ExternalInput").ap()
    out_d = nc.dram_tensor("out", [_HLOC, 8, 128, 258], f32,
                           kind="ExternalOutput").ap()

    cbf_d = nc.dram_tensor("cbf", [128, _NCHUNK * 128], bf16,
                           kind="ExternalInput").ap()

    with tile.TileContext(nc) as tc:
        import contextlib
        with contextlib.ExitStack() as ctx:
            pools = {
                "psP": ctx.enter_context(
                    tc.tile_pool(name="psP", bufs=3, space="PSUM")),
                "psM": ctx.enter_context(
                    tc.tile_pool(name="psM", bufs=2, space="PSUM")),
                "big": ctx.enter_context(tc.tile_pool(name="big", bufs=2)),
                "sm": ctx.enter_context(tc.tile_pool(name="sm", bufs=2)),
                "ptp": ctx.enter_context(tc.tile_pool(name="ptp", bufs=8)),
                "outp": ctx.enter_context(tc.tile_pool(name="outp", bufs=6)),
                "constp": ctx.enter_context(
                    tc.tile_pool(name="constp", bufs=1)),
            }
            cp = pools["constp"]
            # all mask constants are static -- DMA'd from the host
            indall = cp.tile([128, _NCHUNK * 128], bf16, tag="indall")
            cf32 = cp.tile([128, 353], f32, tag="cf32")
            nc.sync.dma_start(cf32[:], cf32_d)
            tri128 = cf32[:, 0:128]
            causal_add = cf32[0:32, 128:160]
            causal01 = cf32[0:32, 160:192]
            eye01 = cf32[0:32, 192:224]
            nshift = cf32[:, 224:225]
            consts = dict(indall=indall, tri128=tri128,
                          causal_add=causal_add, causal01=causal01,
                          eye01=eye01, nshift=nshift,
                          csts=cf32, cbf_d=cbf_d)
            _emit(nc, tc, pools, consts, qT_d, kT_d, va_d, out_d, mybir)
    nc.compile()
    return nc


_lock = threading.Lock()
_cached_nc = None


def _get_nc():
    global _cached_nc
    with _lock:
        if _cached_nc is None:
            _cached_nc = build_nc()
    return _cached_nc


_HOST_CONSTS = None


def _host_consts():
    global _HOST_CONSTS
    if _HOST_CONSTS is None:
        import ml_dtypes
        bf = ml_dtypes.bfloat16
        # indall[b, ci*128 + kj] = 1.0 iff b == 2*ci + kj//64 (b < 32)
        ind = np.zeros((128, _NCHUNK * 128), dtype=np.float32)
        b = np.arange(128)[:, None]
        col = np.arange(_NCHUNK * 128)[None, :]
        ind[(b == 2 * (col // 128) + (col % 128) // 64)] = 1.0
        cf32 = np.zeros((128, 353), dtype=np.float32)
        p = np.arange(128)[:, None]
        f = np.arange(128)[None, :]
        cf32[:, 0:128] = np.where(f >= p, 0.0, -_BIGM)        # tri128
        qb = np.arange(32)[:, None]
        kb = np.arange(32)[None, :]
        cf32[0:32, 128:160] = np.where(kb <= qb, 0.0, _NEG_BL)  # causal_add
        cf32[0:32, 160:192] = (kb <= qb).astype(np.float32)     # causal01
        cf32[0:32, 192:224] = np.eye(32, dtype=np.float32)      # eye01
        cf32[:, 224] = -_SHIFT                                  # nshift
        _HOST_CONSTS = (np.ascontiguousarray(ind.astype(bf)), cf32)
    return _HOST_CONSTS


def make_in_maps(q, k, v):
    import ml_dtypes
    bf = ml_dtypes.bfloat16
    q = np.asarray(q, dtype=np.float32)
    k = np.asarray(k, dtype=np.float32)
    v = np.asarray(v, dtype=np.float32)
    cbf, cf32_base = _host_consts()
    # block sums for the adaptive mask, precomputed per head: qbT [D, NB]
    # q block sums; kbs [D, NB] k block sums with the per-head k sequence
    # mean removed (smooth_k)
    qb = q[0].reshape(_H, _NB, _BLK, _D).sum(axis=2)       # [H, NB, D]
    kb = k[0].reshape(_H, _NB, _BLK, _D).sum(axis=2)
    kb = kb - kb.sum(axis=1, keepdims=True) / float(_NB)
    qbT_all = np.ascontiguousarray(qb.transpose(0, 2, 1))  # [H, D, NB]
    kbsT_all = np.ascontiguousarray(kb.transpose(0, 2, 1))
    in_maps = []
    for i in range(_NCORES):
        sl = slice(i * _HLOC, (i + 1) * _HLOC)
        qT = q[0, sl].transpose(0, 2, 1).astype(bf)
        kT = k[0, sl].transpose(0, 2, 1).astype(bf)
        # v shuffled into the exact SBUF tile layout [128, c*129+x] so the
        # DMA is a single clean 2D transfer (128 rows x 4128 B)
        va = np.empty((_HLOC, _S, _D + 1), dtype=bf)
        va[:, :, :_D] = v[0, sl].astype(bf)
        va[:, :, _D] = 1.0
        va = np.ascontiguousarray(
            va.reshape(_HLOC, _NCHUNK, 128, _D + 1).transpose(0, 2, 1, 3)
            .reshape(_HLOC, 128, _NCHUNK * (_D + 1)))
        cf32 = cf32_base.copy()
        cf32[:, 225:257] = qbT_all[sl][0]
        cf32[:, 257:289] = qbT_all[sl][1]
        cf32[:, 289:321] = kbsT_all[sl][0]
        cf32[:, 321:353] = kbsT_all[sl][1]
        in_maps.append({"qT": np.ascontiguousarray(qT),
                        "kT": np.ascontiguousarray(kT),
                        "va": va,
                        "cbf": cbf,
                        "cf32": cf32})
    return in_maps


def kernel(q, k, v):
    from concourse.bass_utils import run_bass_kernel_spmd

    nc = _get_nc()
    in_maps = make_in_maps(q, k, v)
    res = run_bass_kernel_spmd(nc, in_maps, core_ids=list(range(_NCORES)))
    raw = np.concatenate([res.results[i]["out"] for i in range(_NCORES)],
                         axis=0)                      # [H, 8, 128, 258]
    return finish_output(raw)


def finish_output(raw):
    # raw[h, g, p, ts*129 : ts*129+129] = (numerator[0:128], denominator)
    # for query s = g*256 + ts*128 + p
    raw = np.asarray(raw, dtype=np.float32).reshape(_H, 8, 128, 258)
    out = np.empty((_H, 8, 2, 128, _D), dtype=np.float32)
    for ts in range(2):
        blk = raw[:, :, :, ts * 129:ts * 129 + 129]
        out[:, :, ts] = blk[..., :_D] / blk[..., _D:_D + 1]
    return out.reshape(_B, _H, _S, _D)


if __name__ == "__main__":
    rng = np.random.default_rng(0)
    q = rng.standard_normal((_B, _H, _S, _D), dtype=np.float32)
    k = rng.standard_normal((_B, _H, _S, _D), dtype=np.float32)
    v = rng.standard_normal((_B, _H, _S, _D), dtype=np.float32)
    o = kernel(q, k, v)
    print(o.shape, o.dtype, np.abs(o).max())


# revision 98
# speedup vs baseline: 1.2022x; 1.2022x over previous
"""AdaptiveSparseAttention Trainium2 kernel (8-core head-parallel).

Problem: B=1, H=16, S=2048, D=128 fp32, causal attention with an adaptive
block mask: mean-pool Q/K per 64-block, softmax block scores, keep the
minimal top-p (0.95) set of key blocks per query block (plus diagonal).

Sharding: 2 heads per NeuronCore, fully local (no collectives).

Host-side prep (numpy, outside the NEFF): q,k shipped pre-transposed
[D, S] bf16; v shipped bf16 pre-chunked into the SBUF tile layout with a
ones column appended (so P@V produces the softmax denominator for free);
per-head Q/K block sums (with the k sequence-mean removed -- smooth_k)
and all static mask constants packed into two small tensors; the final
numerator/denominator divide and output transpose also run on the host.
smooth_k is dropped from the main logits path entirely: subtracting the
per-head K mean shifts every logit of a softmax row by a per-query
constant, which softmax cancels exactly.

Device program (per head, heads sequential through one software-
pipelined wave loop):
  - [32,32] f32 block-score chain reproduces the reference
    argsort/cumsum top-p construction exactly (no ties): causal-masked
    block logits, unnormalized softmax (fixed e^-9 shift -- the top-p
    ratio test is scale-invariant), pairwise strictly-greater mass
    comparison against tau*rowsum, diagonal forced on.  The keep mask is
    expanded to negk[kb, qi] (-1e9 where dropped, bf16, 128 partitions
    with ones-padded keepT so rows 32:128 are exact zeros).
  - flash attention over groups of 512 queries in [128, 1024] f32 psum
    waves (2 kj-chunks x 512 queries): per chunk ONE QK matmul
    LT[kj,qi] = kT.T @ qT (bf16, leading above-diagonal 128-blocks
    skipped) immediately followed by the block-mask add as a second
    matmul (indall.T @ negk, K=128 so LDWEIGHTS pipelining never
    alternates); token-level causal via [128,128] triangular DVE adds on
    diagonal chunks; exp on ScalarE (scale=1/sqrt(D), bias=-9) -> PT
    bf16; P@V accumulates two full-bank [128,512] psum tiles per group
    (2x 129 cols each; only the very first matmul into a tile sets
    start=True -- start clears has_written for the WHOLE bank).
  - emission is software-pipelined two waves ahead (QK of wave i+2
    issues right after exp of wave i) so the PE streams back-to-back;
    input DMAs are split across both hwdge rings (sync + scalar) in
    first-use order, with bulk transfers dispatched after the head-0
    mask chain so they never delay it.
"""

import math
import threading

import numpy as np

_B, _H, _S, _D = 1, 16, 2048, 128
_NCORES = 8
_HLOC = _H // _NCORES  # heads per core
_BLK = 64
_NB = _S // _BLK       # 32 key/query blocks
_TAU = 0.95
_SCALE = 1.0 / math.sqrt(_D)
_SHIFT = 9.0           # constant softmax shift; |scaled logits| < ~6
_BIGM = 1.0e9          # additive mask magnitude (pre-scale), diag tri only
_NEG_BL = -1.0e30      # block-logit causal mask value (matches reference)

_NCHUNK = _S // 128    # 16 sequence chunks of 128


class _Head:
    pass


def _emit(nc, tc, pools, consts, qT_d, kT_d, va_d, out_d, mybir):
    f32 = mybir.dt.float32
    bf16 = mybir.dt.bfloat16
    AF = mybir.ActivationFunctionType
    OP = mybir.AluOpType
    AX = mybir.AxisListType

    psP = pools["psP"]
    psM = pools["psM"]
    big = pools["big"]
    sm = pools["sm"]
    ptp = pools["ptp"]
    outp = pools["outp"]

    indall = consts["indall"]
    tri128 = consts["tri128"]
    causal_add = consts["causal_add"]
    causal01 = consts["causal01"]
    eye01 = consts["eye01"]
    nshift = consts["nshift"]

    heads = []
    for h in range(_HLOC):
        H = _Head()
        H.h = h
        H.qT = big.tile([128, _S], bf16, tag="qT", name=f"qT{h}")
        H.kT = big.tile([128, _S], bf16, tag="kT", name=f"kT{h}")
        H.va = big.tile([128, _NCHUNK * 129], bf16, tag="va", name=f"va{h}")
        H.va3 = H.va[:].rearrange("p (c x) -> p c x", x=129)
        heads.append(H)

    # ---- input DMAs (sync ring).  The tiny block-sum tensors go FIRST
    # so the mask chain (which gates the first wave's mask matmul) starts
    # immediately; then head0 k/q (first QK waves), head0 v (first PV),
    # then head1.
    for H in heads:
        h = H.h
        H.qbT = consts["csts"][:, 225 + h * 32:225 + h * 32 + 32]
        H.kbs = consts["csts"][:, 289 + h * 32:289 + h * 32 + 32]
    # Two DMA rings (sync + scalar), transfers ordered by first use so the
    # wave pipeline ramps while the bulk of the input still streams in.
    # Only the head-0 critical pieces are dispatched before the chain --
    # scalar-ring dispatches occupy the ACT queue and must not delay the
    # chain's block-softmax exp.
    H0, H1 = heads
    nc.sync.dma_start(H0.kT[:, 0:1024], kT_d[0][:, 0:1024])
    nc.scalar.dma_start(H0.qT[:, 0:1024], qT_d[0][:, 0:1024])
    nc.scalar.dma_start(indall[:], consts["cbf_d"])
    nc.sync.dma_start(H0.kT[:, 1024:2048], kT_d[0][:, 1024:2048])

    def emit_bulk_dma():
        nc.sync.dma_start(H0.qT[:, 1024:2048], qT_d[0][:, 1024:2048])
        nc.scalar.dma_start(H0.va[:], va_d[0])
        nc.sync.dma_start(H1.kT[:], kT_d[1])
        nc.scalar.dma_start(H1.qT[:], qT_d[1])
        nc.scalar.dma_start(H1.va[:], va_d[1])

    # ---- block-score keep chain, split into parts for sprinkling ----
    # qbT ([128,32] f32 q block sums) and kbs (smoothed k block sums)
    # arrive precomputed from the host
    def chain_b(H):
        h = H.h
        H.blp = psP.tile([32, 32], f32, tag="lt", name=f"blp{h}")
        nc.tensor.matmul(H.blp[:], H.qbT[:], H.kbs[:], start=True, stop=True)

    def chain_c(H):
        h = H.h
        bl = sm.tile([32, 32], f32, tag="bl", name=f"bl{h}")
        nc.vector.scalar_tensor_tensor(
            bl[:], H.blp[:], _SCALE / float(_BLK * _BLK), causal_add,
            op0=OP.mult, op1=OP.add)
        # unnormalized block softmax with a fixed shift: block logits are
        # bounded (|bl| << 1), and the keep test below compares the sum of
        # strictly-greater exps against tau * rowsum -- invariant to the
        # uniform e^-9 scaling
        H.bp = sm.tile([32, 32], f32, tag="bp", name=f"bp{h}")
        rs = sm.tile([32, 1], f32, tag="rs", name=f"rs{h}")
        nc.scalar.activation(H.bp[:], bl[:], AF.Exp, bias=nshift[0:32],
                             scale=1.0, accum_out=rs[:])
        H.taurs = sm.tile([32, 1], f32, tag="taurs", name=f"taurs{h}")
        nc.vector.tensor_scalar_mul(H.taurs[:], rs[:], _TAU)

    def chain_d(H):
        h = H.h
        a_ap = H.bp[:].unsqueeze(1).broadcast_to((32, 32, 32))
        b_ap = H.bp[:].unsqueeze(2).broadcast_to((32, 32, 32))
        gt = sm.tile([32, 32 * 32], f32, tag="gt", name=f"gt{h}")
        H.gt3 = gt[:].rearrange("p (a b) -> p a b", a=32)
        nc.vector.tensor_tensor(H.gt3, a_ap, b_ap, op=OP.is_gt)

    def chain_e(H):
        h = H.h
        a_ap = H.bp[:].unsqueeze(1).broadcast_to((32, 32, 32))
        pr = sm.tile([32, 32 * 32], f32, tag="pr", name=f"pr{h}")
        pr3 = pr[:].rearrange("p (a b) -> p a b", a=32)
        nc.vector.tensor_tensor(pr3, H.gt3, a_ap, op=OP.mult)
        H.tt = sm.tile([32, 32], f32, tag="tt", name=f"tt{h}")
        nc.vector.reduce_sum(H.tt[:], pr3, axis=AX.X)

    def chain_f(H):
        h = H.h
        keep = sm.tile([32, 32], f32, tag="keep", name=f"keep{h}")
        nc.vector.scalar_tensor_tensor(
            keep[:], H.tt[:], H.taurs[:], causal01,
            op0=OP.is_lt, op1=OP.mult)
        nc.vector.tensor_tensor(keep[:], keep[:], eye01, op=OP.max)
        # keepT padded to 128 partitions with ones (rows 32:128 -> negk 0)
        H.keepT = sm.tile([128, 32], f32, tag="keepT", name=f"keepT{h}")
        nc.vector.memset(H.keepT[32:64, :], 1.0)
        nc.vector.memset(H.keepT[64:96, :], 1.0)
        nc.vector.memset(H.keepT[96:128, :], 1.0)
        nc.vector.transpose(H.keepT[0:32, :], keep[:])

    def sel_dve(g, w0, ci):
        # offloading mask adds to DVE balances engine busy-time on paper
        # (PE 49 / ACT 48 / DVE 46) but couples exp behind the in-order
        # DVE queue and loses ~3us to per-wave stalls -- keep all on PE
        return False

    def chain_i(H):
        # kexp[p, ci*32+qb] = -BIGM where block (qb, kb=2ci+p//64) dropped
        # (partition expansion of the keep mask via tiny K=128 matmuls
        # against the static indicator; keepm1 rows 32:128 are zero)
        h = H.h
        keepm1 = sm.tile([128, 32], bf16, tag="keepm1", name=f"keepm1{h}")
        nc.vector.tensor_scalar(keepm1[:], H.keepT[:], 1.0, _BIGM,
                                op0=OP.subtract, op1=OP.mult)
        kx = psP.tile([128, 512], f32, tag="lt", name=f"kx{h}")
        for ci in range(_NCHUNK):
            nc.tensor.matmul(
                kx[:, ci * 32:(ci + 1) * 32],
                indall[:, ci * 128:(ci + 1) * 128],
                keepm1[:], start=True, stop=True)
        H.kexp = sm.tile([128, _NCHUNK * 32], bf16, tag="kexp",
                         name=f"kexp{h}")
        nc.vector.tensor_copy(H.kexp[:], kx[:])

    def chain_g(H):
        # negk[kb, qi] = -BIGM where block (qb(qi), kb) is dropped, else 0.
        # Full 128 partitions (rows >= 32 zero, from the ones-padding of
        # keepT) so the mask matmul has K=128 like every other main-loop
        # matmul -- alternating K breaks LDWEIGHTS pipelining.
        # built in two pieces: the first 512 query cols (group 0) on DVE so
        # wave 0's mask matmul can start immediately; the rest on gpsimd in
        # parallel
        h = H.h
        H.negk = sm.tile([128, _S], bf16, tag="negk", name=f"negk{h}")
        nc.vector.tensor_scalar(
            H.negk[:, 0:512].rearrange("p (a b) -> p a b", b=_BLK),
            H.keepT[:, 0:8].unsqueeze(2).broadcast_to((128, 8, _BLK)),
            1.0, _BIGM, op0=OP.subtract, op1=OP.mult)
        nc.vector.tensor_scalar(
            H.negk[:, 512:2048].rearrange("p (a b) -> p a b", b=_BLK),
            H.keepT[:, 8:32].unsqueeze(2).broadcast_to((128, 24, _BLK)),
            1.0, _BIGM, op0=OP.subtract, op1=OP.mult)

    # head0's chain runs to completion FIRST (it gates the first mask
    # matmuls); the bulk DMAs and head1's chain follow
    for fn in (chain_b, chain_c, chain_d, chain_e, chain_f, chain_g):
        fn(H0)
    emit_bulk_dma()
    chain_b(H1)

    # ---- flat software-pipelined wave list over both heads ----
    # groups of 512 queries (4 output t-blocks), waves of 2 kj-chunks
    ngrp = _S // 512
    waves = []  # (H, g, w0, wn)
    for H in heads:
        for g in range(ngrp):
            nch = 4 * g + 4
            for w0 in range(0, nch, 2):
                waves.append((H, g, w0, min(2, nch - w0)))

    def emit_qk(i):
        H, g, w0, wn = waves[i]
        qlo = g * 512
        ltw = psP.tile([128, 1024], f32, tag="lt",
                       name=f"lt{H.h}_{g}_{w0}")
        H.ltw = ltw
        for ci in range(w0, w0 + wn):
            # leading 128-col blocks above the causal diagonal are never
            # consumed -- skip them
            qo = max(0, (ci - 4 * g)) * 128
            sl = ltw[:, (ci - w0) * 512 + qo:(ci - w0) * 512 + 512]
            dve = sel_dve(g, w0, ci)
            nc.tensor.matmul(
                sl, H.kT[:, ci * 128:(ci + 1) * 128],
                H.qT[:, qlo + qo:qlo + 512], start=True, stop=dve)
            if not dve:
                nc.tensor.matmul(
                    sl, indall[:, ci * 128:(ci + 1) * 128],
                    H.negk[:, qlo + qo:qlo + 512], start=False, stop=True,
                    skip_group_check=True)
        return ltw

    lt_of = {}
    lt_of[0] = emit_qk(0)
    lt_of[1] = emit_qk(1)
    for fn in (chain_c, chain_d, chain_e, chain_f, chain_g):
        fn(H1)

    for i, (H, g, w0, wn) in enumerate(waves):
        ltw = lt_of.pop(i)
        h = H.h
        # token-level causal on diagonal chunks
        for ci in range(max(w0, 4 * g), w0 + wn):
            off = (ci - w0) * 512 + (ci - 4 * g) * 128
            nc.vector.tensor_tensor(
                ltw[:, off:off + 128], ltw[:, off:off + 128],
                tri128, op=OP.add)
        # DVE-side block-mask adds for the offloaded chunks
        for ci in range(w0, w0 + wn):
            if not sel_dve(g, w0, ci):
                continue
            sl3 = ltw[:, (ci - w0) * 512:(ci - w0) * 512 + 512].rearrange(
                "p (a b) -> p a b", b=_BLK)
            kb = H.kexp[:, ci * 32 + 8 * g:ci * 32 + 8 * g + 8]
            nc.vector.tensor_tensor(
                sl3, sl3, kb.unsqueeze(2).broadcast_to((128, 8, _BLK)),
                op=OP.add)
        # QK of wave i+2 issues before exp(i) in queue order: its psum
        # slot was already freed by exp(i-1), so PE starts it while ACT
        # works through exp(i)
        if i + 2 < len(waves):
            lt_of[i + 2] = emit_qk(i + 2)
        ptw = ptp.tile([128, 1024], bf16, tag="pt",
                       name=f"pt{h}_{g}_{w0}")
        nc.scalar.activation(ptw[:, 0:wn * 512], ltw[:, 0:wn * 512],
                             AF.Exp, bias=nshift, scale=_SCALE)
        if w0 == 0:
            # two full-bank accumulator tiles per 512-query group (psum
            # bank = 512 f32); start=True clears has_written for the
            # WHOLE bank, so only the very first matmul into each tile
            # may set it
            H.acc_lo = psM.tile([128, 512], f32, tag="acc",
                                name=f"accL{h}_{g}")
            H.acc_hi = psM.tile([128, 512], f32, tag="acc",
                                name=f"accH{h}_{g}")
        for ci in range(w0, w0 + wn):
            for t in range(max(4 * g, ci), 4 * g + 4):
                ts = t - 4 * g
                acc = H.acc_lo if ts < 2 else H.acc_hi
                nc.tensor.matmul(
                    acc[:, (ts % 2) * 129:(ts % 2) * 129 + 129],
                    ptw[:, (ci - w0) * 512 + ts * 128:
                        (ci - w0) * 512 + ts * 128 + 128],
                    H.va3[:, ci, :],
                    start=(ci == 0 and ts % 2 == 0),
                    stop=(ci == 4 * g + 2 * (ts // 2) + 1),
                    skip_group_check=True)
        # ship raw numerator+denominator as soon as each accumulator pair
        # stops; the host does the divide.  acc_lo stops with chunk 4g+1.
        if w0 <= 4 * g + 1 < w0 + wn:
            o = outp.tile([128, 258], f32, tag="o", name=f"o{h}_{g}_0")
            nc.vector.tensor_copy(o[:], H.acc_lo[:, 0:258])
            nc.sync.dma_start(out_d[h, 2 * g], o[:])
        if w0 + wn == 4 * g + 4:
            o = outp.tile([128, 258], f32, tag="o", name=f"o{h}_{g}_1")
            nc.vector.tensor_copy(o[:], H.acc_hi[:, 0:258])
            nc.sync.dma_start(out_d[h, 2 * g + 1], o[:])


def build_nc():
    import concourse.mybir as mybir
    import concourse.tile as tile
    from concourse import bacc
    from concourse.masks import make_identity

    f32 = mybir.dt.float32
    bf16 = mybir.dt.bfloat16
    OP = mybir.AluOpType

    nc = bacc.Bacc("TRN2", target_bir_lowering=False, debug=False,
                   enable_asserts=False, num_devices=_NCORES)
    qT_d = nc.dram_tensor("qT", [_HLOC, _D, _S], bf16,
                          kind="ExternalInput").ap()
    kT_d = nc.dram_tensor("kT", [_HLOC, _D, _S], bf16,
                          kind="ExternalInput").ap()
    va_d = nc.dram_tensor("va", [_HLOC, 128, _NCHUNK * 129], bf16,
                          kind="ExternalInput").ap()
    # one packed f32 tensor: consts [128,225] + qbT [2,128,32] + kbs
    # [2,128,32] -> a single small DMA dispatch
    cf32_d = nc.dram_tensor("cf32", [128, 353], f32,
                            kind="ExternalInput").ap()
    out_d = nc.dram_tensor("out", [_HLOC, 8, 128, 258], f32,
                           kind="ExternalOutput").ap()

    cbf_d = nc.dram_tensor("cbf", [128, _NCHUNK * 128], bf16,
                           kind="ExternalInput").ap()

    with tile.TileContext(nc) as tc:
        import contextlib
        with contextlib.ExitStack() as ctx:
            pools = {
                "psP": ctx.enter_context(
                    tc.tile_pool(name="psP", bufs=3, space="PSUM")),
                "psM": ctx.enter_context(
                    tc.tile_pool(name="psM", bufs=2, space="PSUM")),
                "big": ctx.enter_context(tc.tile_pool(name="big", bufs=2)),
                "sm": ctx.enter_context(tc.tile_pool(name="sm", bufs=2)),
                "ptp": ctx.enter_context(tc.tile_pool(name="ptp", bufs=8)),
                "outp": ctx.enter_context(tc.tile_pool(name="outp", bufs=6)),
                "constp": ctx.enter_context(
                    tc.tile_pool(name="constp", bufs=1)),
            }
            cp = pools["constp"]
            # all mask constants are static -- DMA'd from the host
            indall = cp.tile([128, _NCHUNK * 128], bf16, tag="indall")
            cf32 = cp.tile([128, 353], f32, tag="cf32")
            nc.sync.dma_start(cf32[:], cf32_d)
            tri128 = cf32[:, 0:128]
            causal_add = cf32[0:32, 128:160]
            causal01 = cf32[0:32, 160:192]
            eye01 = cf32[0:32, 192:224]
            nshift = cf32[:, 224:225]
            consts = dict(indall=indall, tri128=tri128,
                          causal_add=causal_add, causal01=causal01,
                          eye01=eye01, nshift=nshift,
                          csts=cf32, cbf_d=cbf_d)
            _emit(nc, tc, pools, consts, qT_d, kT_d, va_d, out_d, mybir)
    nc.compile()
    return nc


_lock = threading.Lock()
_cached_nc = None


def _get_nc():
    global _cached_nc
    with _lock:
        if _cached_nc is None:
            _cached_nc = build_nc()
    return _cached_nc


_HOST_CONSTS = None


def _host_consts():
    global _HOST_CONSTS
    if _HOST_CONSTS is None:
        import ml_dtypes
        bf = ml_dtypes.bfloat16
        # indall[b, ci*128 + kj] = 1.0 iff b == 2*ci + kj//64 (b < 32)
        ind = np.zeros((128, _NCHUNK * 128), dtype=np.float32)
        b = np.arange(128)[:, None]
        col = np.arange(_NCHUNK * 128)[None, :]
        ind[(b == 2 * (col // 128) + (col % 128) // 64)] = 1.0
        cf32 = np.zeros((128, 353), dtype=np.float32)
        p = np.arange(128)[:, None]
        f = np.arange(128)[None, :]
        cf32[:, 0:128] = np.where(f >= p, 0.0, -_BIGM)        # tri128
        qb = np.arange(32)[:, None]
        kb = np.arange(32)[None, :]
        cf32[0:32, 128:160] = np.where(kb <= qb, 0.0, _NEG_BL)  # causal_add
        cf32[0:32, 160:192] = (kb <= qb).astype(np.float32)     # causal01
        cf32[0:32, 192:224] = np.eye(32, dtype=np.float32)      # eye01
        cf32[:, 224] = -_SHIFT                                  # nshift
        _HOST_CONSTS = (np.ascontiguousarray(ind.astype(bf)), cf32)
    return _HOST_CONSTS


def make_in_maps(q, k, v):
    import ml_dtypes
    bf = ml_dtypes.bfloat16
    q = np.asarray(q, dtype=np.float32)
    k = np.asarray(k, dtype=np.float32)
    v = np.asarray(v, dtype=np.float32)
    cbf, cf32_base = _host_consts()
    # block sums for the adaptive mask, precomputed per head: qbT [D, NB]
    # q block sums; kbs [D, NB] k block sums with the per-head k sequence
    # mean removed (smooth_k)
    qb = q[0].reshape(_H, _NB, _BLK, _D).sum(axis=2)       # [H, NB, D]
    kb = k[0].reshape(_H, _NB, _BLK, _D).sum(axis=2)
    kb = kb - kb.sum(axis=1, keepdims=True) / float(_NB)
    qbT_all = np.ascontiguousarray(qb.transpose(0, 2, 1))  # [H, D, NB]
    kbsT_all = np.ascontiguousarray(kb.transpose(0, 2, 1))
    in_maps = []
    for i in range(_NCORES):
        sl = slice(i * _HLOC, (i + 1) * _HLOC)
        qT = q[0, sl].transpose(0, 2, 1).astype(bf)
        kT = k[0, sl].transpose(0, 2, 1).astype(bf)
        # v shuffled into the exact SBUF tile layout [128, c*129+x] so the
        # DMA is a single clean 2D transfer (128 rows x 4128 B)
        va = np.empty((_HLOC, _S, _D + 1), dtype=bf)
        va[:, :, :_D] = v[0, sl].astype(bf)
        va[:, :, _D] = 1.0
        va = np.ascontiguousarray(
            va.reshape(_HLOC, _NCHUNK, 128, _D + 1).transpose(0, 2, 1, 3)
            .reshape(_HLOC, 128, _NCHUNK * (_D + 1)))
        cf32 = cf32_base.copy()
        cf32[:, 225:257] = qbT_all[sl][0]
        cf32[:, 257:289] = qbT_all[sl][1]
        cf32[:, 289:321] = kbsT_all[sl][0]
        cf32[:, 321:353] = kbsT_all[sl][1]
        in_maps.append({"qT": np.ascontiguousarray(qT),
                        "kT": np.ascontiguousarray(kT),
                        "va": va,
                        "cbf": cbf,
                        "cf32": cf32})
    return in_maps


def kernel(q, k, v):
    from concourse.bass_utils import run_bass_kernel_spmd

    nc = _get_nc()
    in_maps = make_in_maps(q, k, v)
    res = run_bass_kernel_spmd(nc, in_maps, core_ids=list(range(_NCORES)))
    raw = np.concatenate([res.results[i]["out"] for i in range(_NCORES)],
                         axis=0)                      # [H, 8, 128, 258]
    return finish_output(raw)


def finish_output(raw):
    # raw[h, g, p, ts*129 : ts*129+129] = (numerator[0:128], denominator)
    # for query s = g*256 + ts*128 + p
    raw = np.asarray(raw, dtype=np.float32).reshape(_H, 8, 128, 258)
    out = np.empty((_H, 8, 2, 128, _D), dtype=np.float32)
    for ts in range(2):
        blk = raw[:, :, :, ts * 129:ts * 129 + 129]
        out[:, :, ts] = blk[..., :_D] / blk[..., _D:_D + 1]
    return out.reshape(_B, _H, _S, _D)


if __name__ == "__main__":
    rng = np.random.default_rng(0)
    q = rng.standard_normal((_B, _H, _S, _D), dtype=np.float32)
    k = rng.standard_normal((_B, _H, _S, _D), dtype=np.float32)
    v = rng.standard_normal((_B, _H, _S, _D), dtype=np.float32)
    o = kernel(q, k, v)
    print(o.shape, o.dtype, np.abs(o).max())
